# revision 2
# baseline (speedup 1.0000x reference)
"""MoE FFN (8 experts, top-2) on 8 Trainium2 NeuronCores.

Strategy: expert parallelism with host-side token routing.
  - Host computes the (tiny) gate: logits = x @ gate_w.T, top-2, softmax.
  - Tokens are gathered per expert and padded to a common capacity C.
  - Core e runs a dense FFN (gelu(x@W1[e].T+b1[e])@W2[e].T+b2[e]) over the
    C tokens routed to expert e, all in one SPMD Bass program.
  - Host scatters y back with the combine weights and sums the two
    expert contributions per token.

Device compute: error-compensated fp8 via DoubleRow matmuls.
  TRN2's fp8(e4m3) DoubleRow mode contracts two 128-deep k-slices per
  instruction at 0.5 cycles per output column - 2x bf16 MACs/cycle.
  Plain e4m3 quantization costs ~3.5% relative error per matmul (fails
  the 2e-2 budget), so each operand is split hi+lo:
      w ~ w_hi + w_lo   (both e4m3, at scale 256; residual ~0.06%)
      x ~ x_hi + x_lo   (both e4m3, at scale 1)
  and each k-pair (2j, 2j+1) issues three DoubleRow instructions:
      A: (w_hi[2j], w_hi[2j+1]) . (x_hi[2j], x_hi[2j+1])
      B: (w_lo[2j], w_lo[2j+1]) . (x_hi[2j], x_hi[2j+1])
      C: (w_hi[2j], w_hi[2j+1]) . (x_lo[2j], x_lo[2j+1])
  which realizes w_hi.x_hi + w_lo.x_hi + w_hi.x_lo per k (only the
  2nd-order w_lo.x_lo term is dropped).  3 instructions replace 2 bf16
  matmuls -> 0.75x tensor cycles at rel err ~2.5e-3 (vs 3.2e-3 bf16).

Per-core layout (per 512-token tile):
  FFN1: psum[m128, tok] += DoubleRow groups over KH/2=4 k-pairs (12 mm)
        ACT#1: g   = gelu(psum/256 + b1)  -> bf16 scratch
        ACT#2: hhi = gelu(psum/256 + b1)  -> e4m3
        DVE:   hlo = g - hhi              -> e4m3
  FFN2: psum[m128, tok] += DoubleRow groups over KI/2=16 k-pairs (48 mm)
        DVE:   y = psum/256 + b2          -> f32, DMA out
Weights resident in SBUF as e4m3 hi|lo pairs; tokens stream in <=512 tiles.
"""

import sys
import types

import numpy as np
import ml_dtypes

import concourse.bass as bass
import concourse.tile as tile
from concourse import mybir
from concourse.bass_utils import run_bass_kernel_spmd
from bass_rust import ScopedClock, VectorClock


def _ensure_axon_hooks():
    """run_bass_kernel_spmd(trace=True) under axon imports antenv.axon_hooks,
    which this image's antenv lacks.  Register an equivalent module backed by
    trn_agent_boot's ctypes NTFF hook so tracing works (and trace=False paths
    are unaffected)."""
    try:
        import antenv.axon_hooks  # noqa: F401
        return
    except ImportError:
        pass
    hook = None
    try:
        from trn_agent_boot.trn_boot import _ntff_profile_via_ctypes
        hook = _ntff_profile_via_ctypes("/opt/axon/libaxon_pjrt.so")
    except Exception:
        hook = None
    mod = types.ModuleType("antenv.axon_hooks")
    _state = {"hook": hook}
    mod.get_axon_ntff_profile_hook = lambda: _state["hook"]
    mod.set_axon_ntff_profile_hook = lambda h: _state.__setitem__("hook", h)
    sys.modules["antenv.axon_hooks"] = mod
    try:
        import antenv
        antenv.axon_hooks = mod
    except ImportError:
        pass


_ensure_axon_hooks()

H = 1024          # hidden
I = 4096          # intermediate
E = 8             # experts
NCORES = 8
BF16 = mybir.dt.bfloat16
F32 = mybir.dt.float32
FP8 = mybir.dt.float8e4
E4NP = ml_dtypes.float8_e4m3   # TRN FP8_EXP4 (max normal 240, has inf)
SW = 256.0                     # weight scale: w stored as e4m3(256*w)
DR = mybir.MatmulPerfMode.DoubleRow
SUB = mybir.AluOpType.subtract
MUL = mybir.AluOpType.mult
ADD = mybir.AluOpType.add


class _TC(tile.TileContext):
    """TileContext whose tail drain splits its sem waits across SP nops.

    The walrus pinned in this container rejects a Drain instruction carrying
    more than a couple of sync waits ("Too many sync wait commands",
    CoreV3GenImpl.cpp:104).  Emit one wait-carrier nop per logical processor
    instead, then a waitless drain.
    """

    def _drain_and_barrier(self, tick_clock, wait_clock):
        nc = self.nc
        gc = tick_clock.global_clock
        ticks = eval(repr(gc).replace("VectorClock(", "").rstrip(")"))
        for i, t in enumerate(ticks):
            if t > 0:
                partial = [0] * len(ticks)
                partial[i] = t
                carrier = nc.sync.nop(nofuse=True, hint=f"drain_wait_{i}")
                wait_clock.add_sem_waits(
                    carrier.ins, ScopedClock({None: VectorClock(partial)})
                )
        nc.sync.drain()
        nc.all_engine_barrier()
        assert self.sems is not None
        popped = nc._tile_sem_poison_stack.pop()
        assert popped is self._sem_poison
        nc.clear_and_free_semaphores(list(self.sems.allocated().values()))
        nc.all_engine_barrier()


def _split_waits(nc, maxw=1):
    """The pinned walrus rejects instructions carrying more than one
    embedded sync wait ("Too many sync wait commands").  Hoist excess waits
    onto freshly inserted same-engine nops placed directly before the
    instruction - the engine sequencer executes them in order, so the
    semantics are identical."""
    for fn in nc.m.functions:
        for bb in fn.blocks:
            new = []
            changed = False
            for inst in bb.instructions:
                si = inst.sync_info
                waits = list(si.on_wait) if si is not None else []
                if len(waits) > maxw:
                    changed = True
                    n_extra = len(waits) - maxw
                    for i in range(0, n_extra, maxw):
                        nop = mybir.InstNoOp(
                            name=nc.get_next_instruction_name(),
                            engine=inst.engine,
                            sync_info=mybir.SyncInfo(
                                on_wait=waits[i:i + maxw], on_update=[]
                            ),
                            bass_nofuse=True,
                        )
                        nc.register_instruction(nop, overwrite=True)
                        new.append(nop)
                    si.on_wait = waits[n_extra:]
                new.append(inst)
            if changed:
                bb.instructions = new


def _token_tiles(C):
    # Remainder tile last: the first (full) tile's FFN1 masks the W2 load.
    tiles = [512] * (C // 512)
    if C % 512:
        tiles.append(C % 512)
    return tiles


def _build(C):
    """Dense per-expert fp8 FFN over C tokens; one SPMD program for all."""
    KH = H // 128   # 8  k-tiles over hidden
    KI = I // 128   # 32 k-tiles over inter
    nc = bass.Bass()
    xhi = nc.declare_dram_parameter("xhi", [H, C], FP8, isOutput=False)
    xlo = nc.declare_dram_parameter("xlo", [H, C], FP8, isOutput=False)
    # [hi | lo] blocks of I (resp. H) columns per 128-row k-slice.
    w1 = nc.declare_dram_parameter("w1", [H, 2 * I], FP8, isOutput=False)
    w2 = nc.declare_dram_parameter("w2", [I, 2 * H], FP8, isOutput=False)
    b1 = nc.declare_dram_parameter("b1", [128, KI], F32, isOutput=False)
    b2 = nc.declare_dram_parameter("b2", [128, KH], F32, isOutput=False)
    yt = nc.declare_dram_parameter("yt", [H, C], F32, isOutput=True)

    with _TC(nc) as tc:
        with (
            tc.tile_pool(name="weights", bufs=1) as wpool,
            tc.tile_pool(name="bias", bufs=1) as bpool,
            tc.tile_pool(name="x", bufs=3) as xpool,
            tc.tile_pool(name="h", bufs=1) as hpool,
            tc.tile_pool(name="g", bufs=3) as gpool,
            tc.tile_pool(name="o", bufs=4) as opool,
            tc.tile_pool(name="ps1", bufs=4, space="PSUM") as ps1pool,
            tc.tile_pool(name="ps2", bufs=4, space="PSUM") as ps2pool,
        ):
            # Latency-critical small loads on GpSimd SWDGE queues so they
            # don't queue behind the 16 MB of weight traffic on the sync
            # HWDGE queues.
            b1s = bpool.tile([128, KI], F32, tag="b1")
            nc.gpsimd.dma_start(b1s[:], b1[:])
            b2s = bpool.tile([128, KH], F32, tag="b2")
            nc.gpsimd.dma_start(b2s[:], b2[:])
            # W1 as [k, hi|lo, I-cols]; delivered in column phases so FFN1
            # can start after the first few hundred KB.
            w1s = wpool.tile([128, KH, 2, I], FP8, tag="w1")
            bounds = [0, 256] + [256 + 960 * i for i in range(1, 5)]
            for lo, hi in zip(bounds[:-1], bounds[1:]):
                for k in range(KH):
                    for s in range(2):
                        nc.sync.dma_start(
                            w1s[:, k, s, lo:hi],
                            w1[k * 128:(k + 1) * 128, s * I + lo:s * I + hi],
                        )
            # W2 afterwards, in FFN2 consumption order (k ascending); hi and
            # lo blocks of one k-slice arrive in a single contiguous DMA.
            w2s = wpool.tile([128, KI, 2, H], FP8, tag="w2")
            for k in range(KI):
                nc.sync.dma_start(
                    w2s[:, k, :, :], w2[k * 128:(k + 1) * 128, :]
                )

            off = 0
            for ti, tw in enumerate(_token_tiles(C)):
                xs = xpool.tile([128, KH, 2, tw], FP8, tag="xt")
                # First tile: halve each chunk so the 8 SWDGE queues turn
                # around faster and the first psum-group unblocks sooner.
                nsplit = 2 if ti == 0 else 1
                for k in range(KH):
                    step = tw // nsplit
                    for sp in range(nsplit):
                        cl, cu = sp * step, (sp + 1) * step
                        nc.gpsimd.dma_start(
                            xs[:, k, 0, cl:cu],
                            xhi[k * 128:(k + 1) * 128, off + cl:off + cu],
                        )
                        nc.gpsimd.dma_start(
                            xs[:, k, 1, cl:cu],
                            xlo[k * 128:(k + 1) * 128, off + cl:off + cu],
                        )
                ht = hpool.tile([128, KI, 2, tw], FP8, tag="h")
                for m in range(KI):
                    ps = ps1pool.tile([128, tw], F32, tag="ps1")
                    ml, mu = m * 128, (m + 1) * 128
                    for j in range(KH // 2):
                        ka, kb = 2 * j, 2 * j + 2
                        whi = w1s[:, ka:kb, 0:1, ml:mu]
                        wlo = w1s[:, ka:kb, 1:2, ml:mu]
                        xh = xs[:, ka:kb, 0:1, :]
                        xl = xs[:, ka:kb, 1:2, :]
                        nc.tensor.matmul(ps[:], whi, xh,
                                         start=(j == 0), stop=False,
                                         perf_mode=DR)
                        nc.tensor.matmul(ps[:], wlo, xh,
                                         start=False, stop=False,
                                         perf_mode=DR)
                        nc.tensor.matmul(ps[:], whi, xl,
                                         start=False,
                                         stop=(j == KH // 2 - 1),
                                         perf_mode=DR)
                    gt = gpool.tile([128, tw], BF16, tag="g")
                    nc.scalar.activation(
                        gt[:], ps[:],
                        mybir.ActivationFunctionType.Gelu,
                        bias=b1s[:, m:m + 1], scale=1.0 / SW,
                    )
                    nc.scalar.activation(
                        ht[:, m, 0, :], ps[:],
                        mybir.ActivationFunctionType.Gelu,
                        bias=b1s[:, m:m + 1], scale=1.0 / SW,
                    )
                    nc.vector.tensor_tensor(
                        ht[:, m, 1, :], gt[:], ht[:, m, 0, :], SUB
                    )
                for m in range(KH):
                    ps = ps2pool.tile([128, tw], F32, tag="ps2")
                    ml, mu = m * 128, (m + 1) * 128
                    for j in range(KI // 2):
                        ka, kb = 2 * j, 2 * j + 2
                        whi = w2s[:, ka:kb, 0:1, ml:mu]
                        wlo = w2s[:, ka:kb, 1:2, ml:mu]
                        hh = ht[:, ka:kb, 0:1, :]
                        hl = ht[:, ka:kb, 1:2, :]
                        nc.tensor.matmul(ps[:], whi, hh,
                                         start=(j == 0), stop=False,
                                         perf_mode=DR)
                        nc.tensor.matmul(ps[:], wlo, hh,
                                         start=False, stop=False,
                                         perf_mode=DR)
                        nc.tensor.matmul(ps[:], whi, hl,
                                         start=False,
                                         stop=(j == KI // 2 - 1),
                                         perf_mode=DR)
                    ot = opool.tile([128, tw], F32, tag="o")
                    nc.vector.tensor_scalar(
                        ot[:], ps[:], 1.0 / SW, b2s[:, m:m + 1], MUL, ADD
                    )
                    nc.scalar.dma_start(
                        yt[m * 128:(m + 1) * 128, off:off + tw], ot[:]
                    )
                off += tw
    _split_waits(nc)
    return nc


def _route(x, gate_w):
    """Host gate: top-2 of 8 logits + softmax over the selected pair."""
    logits = x @ gate_w.T                         # [T, E] f32
    T = logits.shape[0]
    rows = np.arange(T)
    i1 = np.argmax(logits, axis=1)
    v1 = logits[rows, i1]
    masked = logits.copy()
    masked[rows, i1] = -np.inf
    i2 = np.argmax(masked, axis=1)
    v2 = masked[rows, i2]
    # softmax over (v1, v2) with v1 >= v2
    e2 = np.exp(v2 - v1)
    w1 = 1.0 / (1.0 + e2)
    w2 = 1.0 - w1
    return i1, i2, w1.astype(np.float32), w2.astype(np.float32)


def _hilo(a):
    """e4m3 hi/lo split: a ~ hi + lo with ~2nd-order residual."""
    hi = a.astype(E4NP)
    lo = (a - hi.astype(np.float32)).astype(E4NP)
    return hi, lo


def _run(inputs, trace=False):
    hidden_states = np.asarray(inputs["hidden_states"], dtype=np.float32)
    gate_w = np.asarray(inputs["gate_w"], dtype=np.float32)
    W1 = np.asarray(inputs["W1"], dtype=np.float32)
    b1 = np.asarray(inputs["b1"], dtype=np.float32)
    W2 = np.asarray(inputs["W2"], dtype=np.float32)
    b2 = np.asarray(inputs["b2"], dtype=np.float32)

    B, S, _ = hidden_states.shape
    T = B * S
    x = np.ascontiguousarray(hidden_states.reshape(T, H))

    i1, i2, w1, w2 = _route(x, gate_w)
    toks = [np.flatnonzero((i1 == e) | (i2 == e)) for e in range(E)]
    cnts = [len(t) for t in toks]
    C = max(128, -(-max(cnts) // 128) * 128)

    nc = _build(C)

    in_maps = []
    for e in range(E):
        xe = np.zeros((C, H), dtype=np.float32)
        xe[: cnts[e]] = x[toks[e]]
        xeT = np.ascontiguousarray(xe.T)                   # [H, C]
        xh, xl = _hilo(xeT)
        w1h, w1l = _hilo(SW * W1[e].T)                     # [H, I]
        w2h, w2l = _hilo(SW * W2[e].T)                     # [I, H]
        in_maps.append(
            {
                "xhi": xh,
                "xlo": xl,
                "w1": np.ascontiguousarray(
                    np.concatenate([w1h, w1l], axis=1)),   # [H, 2I]
                "w2": np.ascontiguousarray(
                    np.concatenate([w2h, w2l], axis=1)),   # [I, 2H]
                "b1": np.ascontiguousarray(b1[e].reshape(I // 128, 128).T),
                "b2": np.ascontiguousarray(b2[e].reshape(H // 128, 128).T),
            }
        )

    res = run_bass_kernel_spmd(
        nc, in_maps, core_ids=list(range(NCORES)), trace=trace
    )

    out = np.zeros((T, H), dtype=np.float32)
    for e in range(E):
        te = toks[e]
        ye = res.results[e]["yt"][:, : cnts[e]].T          # [cnt, H]
        we = np.where(i1[te] == e, w1[te], w2[te])
        out[te] += we[:, None] * ye
    return out.reshape(B, S, H), res


def kernel(**inputs):
    out, _ = _run(inputs, trace=False)
    return out


# revision 4
# speedup vs baseline: 1.4005x; 1.4005x over previous
"""MoE FFN (8 experts, top-2) on 8 Trainium2 NeuronCores.

Strategy: expert parallelism with host-side token routing.
  - Host computes the (tiny) gate: logits = x @ gate_w.T, top-2, softmax.
  - Tokens are gathered per expert and padded to a common capacity C.
  - Core e runs a dense FFN (gelu(x@W1[e].T+b1[e])@W2[e].T+b2[e]) over the
    C tokens routed to expert e, all in one SPMD Bass program.
  - Host scatters y back with the combine weights and sums the two
    expert contributions per token.

Device kernel layout (per core):
  FFN1: psum[inter128, tok] += W1T[k*128:, m*128:].T @ xT[k*128:, tok]
        h = gelu(psum + b1)           (ACT, writes bf16)
  FFN2: psum[hid128, tok]  += W2T[k*128:, m*128:].T @ h[k*128:, tok]
        y = psum + b2                 (DVE, writes f32)
Weights held resident in SBUF as bf16; tokens stream in tiles of <=512.
"""

import sys
import types

import numpy as np
import ml_dtypes

import concourse.bass as bass
import concourse.tile as tile
from concourse import mybir
from concourse.bass_utils import run_bass_kernel_spmd
from bass_rust import ScopedClock, VectorClock


def _ensure_axon_hooks():
    """run_bass_kernel_spmd(trace=True) under axon imports antenv.axon_hooks,
    which this image's antenv lacks.  Register an equivalent module backed by
    trn_agent_boot's ctypes NTFF hook so tracing works (and trace=False paths
    are unaffected)."""
    try:
        import antenv.axon_hooks  # noqa: F401
        return
    except ImportError:
        pass
    hook = None
    try:
        from trn_agent_boot.trn_boot import _ntff_profile_via_ctypes
        hook = _ntff_profile_via_ctypes("/opt/axon/libaxon_pjrt.so")
    except Exception:
        hook = None
    mod = types.ModuleType("antenv.axon_hooks")
    _state = {"hook": hook}
    mod.get_axon_ntff_profile_hook = lambda: _state["hook"]
    mod.set_axon_ntff_profile_hook = lambda h: _state.__setitem__("hook", h)
    sys.modules["antenv.axon_hooks"] = mod
    try:
        import antenv
        antenv.axon_hooks = mod
    except ImportError:
        pass


_ensure_axon_hooks()

H = 1024          # hidden
I = 4096          # intermediate
E = 8             # experts
NCORES = 8
BF16 = mybir.dt.bfloat16
F32 = mybir.dt.float32


class _TC(tile.TileContext):
    """TileContext whose tail drain splits its sem waits across SP nops.

    The walrus pinned in this container rejects a Drain instruction carrying
    more than a couple of sync waits ("Too many sync wait commands",
    CoreV3GenImpl.cpp:104).  Emit one wait-carrier nop per logical processor
    instead, then a waitless drain.
    """

    def _drain_and_barrier(self, tick_clock, wait_clock):
        nc = self.nc
        gc = tick_clock.global_clock
        ticks = eval(repr(gc).replace("VectorClock(", "").rstrip(")"))
        for i, t in enumerate(ticks):
            if t > 0:
                partial = [0] * len(ticks)
                partial[i] = t
                carrier = nc.sync.nop(nofuse=True, hint=f"drain_wait_{i}")
                wait_clock.add_sem_waits(
                    carrier.ins, ScopedClock({None: VectorClock(partial)})
                )
        nc.sync.drain()
        nc.all_engine_barrier()
        assert self.sems is not None
        popped = nc._tile_sem_poison_stack.pop()
        assert popped is self._sem_poison
        nc.clear_and_free_semaphores(list(self.sems.allocated().values()))
        nc.all_engine_barrier()


def _split_waits(nc, maxw=1):
    """The pinned walrus rejects instructions carrying more than one
    embedded sync wait ("Too many sync wait commands").  Hoist excess waits
    onto freshly inserted same-engine nops placed directly before the
    instruction — the engine sequencer executes them in order, so the
    semantics are identical."""
    for fn in nc.m.functions:
        for bb in fn.blocks:
            new = []
            changed = False
            for inst in bb.instructions:
                si = inst.sync_info
                waits = list(si.on_wait) if si is not None else []
                if len(waits) > maxw:
                    changed = True
                    n_extra = len(waits) - maxw
                    for i in range(0, n_extra, maxw):
                        nop = mybir.InstNoOp(
                            name=nc.get_next_instruction_name(),
                            engine=inst.engine,
                            sync_info=mybir.SyncInfo(
                                on_wait=waits[i:i + maxw], on_update=[]
                            ),
                            bass_nofuse=True,
                        )
                        nc.register_instruction(nop, overwrite=True)
                        new.append(nop)
                    si.on_wait = waits[n_extra:]
                new.append(inst)
            if changed:
                bb.instructions = new


def _token_tiles(C):
    # Remainder tile last: the first (full) tile's FFN1 masks the W2 load.
    tiles = [512] * (C // 512)
    if C % 512:
        tiles.append(C % 512)
    return tiles


def _build(C):
    """Dense per-expert FFN over C tokens; one SPMD program for all cores."""
    KH = H // 128   # 8  k-tiles over hidden
    KI = I // 128   # 32 k-tiles over inter
    nc = bass.Bass()
    xt = nc.declare_dram_parameter("xt", [H, C], BF16, isOutput=False)
    w1t = nc.declare_dram_parameter("w1t", [H, I], BF16, isOutput=False)
    w2t = nc.declare_dram_parameter("w2t", [I, H], BF16, isOutput=False)
    b1 = nc.declare_dram_parameter("b1", [128, KI], F32, isOutput=False)
    b2 = nc.declare_dram_parameter("b2", [128, KH], F32, isOutput=False)
    yt = nc.declare_dram_parameter("yt", [H, C], F32, isOutput=True)

    tiles = _token_tiles(C)
    offs = [sum(tiles[:i]) for i in range(len(tiles))]

    with _TC(nc) as tc:
        with (
            tc.tile_pool(name="weights", bufs=1) as wpool,
            tc.tile_pool(name="bias", bufs=1) as bpool,
            tc.tile_pool(name="x", bufs=3) as xpool,
            tc.tile_pool(name="h", bufs=1) as hpool,
            tc.tile_pool(name="o", bufs=4) as opool,
            tc.tile_pool(name="ps1", bufs=4, space="PSUM") as ps1pool,
            tc.tile_pool(name="ps2", bufs=4, space="PSUM") as ps2pool,
        ):
            # Both HWDGE queues (SP + Act) are used for delivery.  The Act
            # queue carries the latency-critical startup set (biases + first
            # x tile) first, then the odd W1 phases; SP carries the even W1
            # phases then all of W2.  (The single gpsimd SWDGE queue the
            # previous version used serialized ~18 software-built
            # descriptors in front of the first matmul - 12.6us to MM#0.)
            b1s = bpool.tile([128, KI], F32, tag="b1")
            nc.scalar.dma_start(b1s[:], b1[:])
            b2s = bpool.tile([128, KH], F32, tag="b2")
            nc.scalar.dma_start(b2s[:], b2[:])

            xss = {}

            def emit_x(ti, eng):
                xs = xpool.tile([128, KH * tiles[ti]], BF16, tag="xt",
                                name=f"xs_{ti}")
                tw, off = tiles[ti], offs[ti]
                for k in range(KH):
                    eng.dma_start(
                        xs[:, k * tw:(k + 1) * tw],
                        xt[k * 128:(k + 1) * 128, off:off + tw],
                    )
                xss[ti] = xs

            emit_x(0, nc.scalar)

            w1s = [
                wpool.tile([128, I], BF16, tag=f"w1_{k}", name=f"w1_{k}")
                for k in range(KH)
            ]
            # 256-col W1 phases, alternating SP/Act: delivery on two queues
            # outruns FFN1 consumption (256 KB per 1.7us m-group).
            bounds = list(range(0, I + 1, 256))
            for pi, (lo, hi) in enumerate(zip(bounds[:-1], bounds[1:])):
                eng = nc.sync if pi % 2 == 0 else nc.scalar
                for k in range(KH):
                    eng.dma_start(
                        w1s[k][:, lo:hi], w1t[k * 128:(k + 1) * 128, lo:hi]
                    )
            # W2 afterwards on SP, in FFN2 consumption order (k ascending).
            w2s = []
            for k in range(KI):
                w = wpool.tile([128, H], BF16, tag=f"w2_{k}")
                nc.sync.dma_start(w[:], w2t[k * 128:(k + 1) * 128, :])
                w2s.append(w)

            for ti, tw in enumerate(tiles):
                off = offs[ti]
                xs = xss[ti]
                ht = hpool.tile([128, KI * tw], BF16, tag="h")
                for m in range(KI):
                    # Prefetch the next tile's x mid-FFN1: the Act engine
                    # reaches these dma_starts ~27us into this tile, far
                    # ahead of the next tile's first psum group.
                    if m == KI // 2 and ti + 1 < len(tiles):
                        emit_x(ti + 1, nc.scalar)
                    ps = ps1pool.tile([128, tw], F32, tag="ps1")
                    for k in range(KH):
                        nc.tensor.matmul(
                            ps[:],
                            w1s[k][:, m * 128:(m + 1) * 128],
                            xs[:, k * tw:(k + 1) * tw],
                            start=(k == 0),
                            stop=(k == KH - 1),
                        )
                    nc.scalar.activation(
                        ht[:, m * tw:(m + 1) * tw],
                        ps[:],
                        mybir.ActivationFunctionType.Gelu,
                        bias=b1s[:, m:m + 1],
                    )
                for m in range(KH):
                    ps = ps2pool.tile([128, tw], F32, tag="ps2")
                    for k in range(KI):
                        nc.tensor.matmul(
                            ps[:],
                            w2s[k][:, m * 128:(m + 1) * 128],
                            ht[:, k * tw:(k + 1) * tw],
                            start=(k == 0),
                            stop=(k == KI - 1),
                        )
                    ot = opool.tile([128, tw], F32, tag="o")
                    nc.vector.tensor_scalar_add(ot[:], ps[:], b2s[:, m:m + 1])
                    nc.scalar.dma_start(
                        yt[m * 128:(m + 1) * 128, off:off + tw], ot[:]
                    )
    _split_waits(nc)
    return nc


def _route(x, gate_w):
    """Host gate: top-2 of 8 logits + softmax over the selected pair."""
    logits = x @ gate_w.T                         # [T, E] f32
    T = logits.shape[0]
    rows = np.arange(T)
    i1 = np.argmax(logits, axis=1)
    v1 = logits[rows, i1]
    masked = logits.copy()
    masked[rows, i1] = -np.inf
    i2 = np.argmax(masked, axis=1)
    v2 = masked[rows, i2]
    # softmax over (v1, v2) with v1 >= v2
    e2 = np.exp(v2 - v1)
    w1 = 1.0 / (1.0 + e2)
    w2 = 1.0 - w1
    return i1, i2, w1.astype(np.float32), w2.astype(np.float32)


def _run(inputs, trace=False):
    hidden_states = np.asarray(inputs["hidden_states"], dtype=np.float32)
    gate_w = np.asarray(inputs["gate_w"], dtype=np.float32)
    W1 = np.asarray(inputs["W1"], dtype=np.float32)
    b1 = np.asarray(inputs["b1"], dtype=np.float32)
    W2 = np.asarray(inputs["W2"], dtype=np.float32)
    b2 = np.asarray(inputs["b2"], dtype=np.float32)

    B, S, _ = hidden_states.shape
    T = B * S
    x = np.ascontiguousarray(hidden_states.reshape(T, H))

    i1, i2, w1, w2 = _route(x, gate_w)
    toks = [np.flatnonzero((i1 == e) | (i2 == e)) for e in range(E)]
    cnts = [len(t) for t in toks]
    C = max(128, -(-max(cnts) // 128) * 128)

    nc = _build(C)

    in_maps = []
    for e in range(E):
        xe = np.zeros((C, H), dtype=ml_dtypes.bfloat16)
        xe[: cnts[e]] = x[toks[e]].astype(ml_dtypes.bfloat16)
        in_maps.append(
            {
                "xt": np.ascontiguousarray(xe.T),
                "w1t": np.ascontiguousarray(W1[e].astype(ml_dtypes.bfloat16).T),
                "w2t": np.ascontiguousarray(W2[e].astype(ml_dtypes.bfloat16).T),
                "b1": np.ascontiguousarray(b1[e].reshape(I // 128, 128).T),
                "b2": np.ascontiguousarray(b2[e].reshape(H // 128, 128).T),
            }
        )

    res = run_bass_kernel_spmd(
        nc, in_maps, core_ids=list(range(NCORES)), trace=trace
    )

    out = np.zeros((T, H), dtype=np.float32)
    for e in range(E):
        te = toks[e]
        ye = res.results[e]["yt"][:, : cnts[e]].T          # [cnt, H]
        we = np.where(i1[te] == e, w1[te], w2[te])
        out[te] += we[:, None] * ye
    return out.reshape(B, S, H), res


def kernel(**inputs):
    out, _ = _run(inputs, trace=False)
    return out



# revision 9
# speedup vs baseline: 1.5033x; 1.0734x over previous
"""MoE FFN (8 experts, top-2) on 8 Trainium2 NeuronCores.

Strategy: expert parallelism with host-side token routing.
  - Host computes the (tiny) gate: logits = x @ gate_w.T, top-2, softmax.
  - Tokens are gathered per expert and padded to a common capacity C.
  - Core e runs a dense FFN (gelu(x@W1[e].T+b1[e])@W2[e].T+b2[e]) over the
    C tokens routed to expert e, all in one SPMD Bass program.
  - Host scatters y back with the combine weights and sums the two
    expert contributions per token.

Device kernel layout (per core):
  FFN1: psum[inter128, tok] += W1T[k*128:, m*128:].T @ xT[k*128:, tok]
        h = gelu(psum + b1)           (ACT, writes bf16)
  FFN2: psum[hid128, tok]  += W2T[k*128:, m*128:].T @ h[k*128:, tok]
        y = psum + b2                 (DVE, writes f32)
Weights held resident in SBUF as bf16; tokens stream in tiles of <=512.
"""

import sys
import types

import numpy as np
import ml_dtypes

import concourse.bass as bass
import concourse.tile as tile
from concourse import mybir
from concourse.bass_utils import run_bass_kernel_spmd
from bass_rust import ScopedClock, VectorClock


def _ensure_axon_hooks():
    """run_bass_kernel_spmd(trace=True) under axon imports antenv.axon_hooks,
    which this image's antenv lacks.  Register an equivalent module backed by
    trn_agent_boot's ctypes NTFF hook so tracing works (and trace=False paths
    are unaffected)."""
    try:
        import antenv.axon_hooks  # noqa: F401
        return
    except ImportError:
        pass
    hook = None
    try:
        from trn_agent_boot.trn_boot import _ntff_profile_via_ctypes
        hook = _ntff_profile_via_ctypes("/opt/axon/libaxon_pjrt.so")
    except Exception:
        hook = None
    mod = types.ModuleType("antenv.axon_hooks")
    _state = {"hook": hook}
    mod.get_axon_ntff_profile_hook = lambda: _state["hook"]
    mod.set_axon_ntff_profile_hook = lambda h: _state.__setitem__("hook", h)
    sys.modules["antenv.axon_hooks"] = mod
    try:
        import antenv
        antenv.axon_hooks = mod
    except ImportError:
        pass


_ensure_axon_hooks()

H = 1024          # hidden
I = 4096          # intermediate
E = 8             # experts
NCORES = 8
BF16 = mybir.dt.bfloat16
F32 = mybir.dt.float32


class _TC(tile.TileContext):
    """TileContext whose tail drain splits its sem waits across SP nops.

    The walrus pinned in this container rejects a Drain instruction carrying
    more than a couple of sync waits ("Too many sync wait commands",
    CoreV3GenImpl.cpp:104).  Emit one wait-carrier nop per logical processor
    instead, then a waitless drain.
    """

    def _drain_and_barrier(self, tick_clock, wait_clock):
        nc = self.nc
        gc = tick_clock.global_clock
        ticks = eval(repr(gc).replace("VectorClock(", "").rstrip(")"))
        for i, t in enumerate(ticks):
            if t > 0:
                partial = [0] * len(ticks)
                partial[i] = t
                carrier = nc.sync.nop(nofuse=True, hint=f"drain_wait_{i}")
                wait_clock.add_sem_waits(
                    carrier.ins, ScopedClock({None: VectorClock(partial)})
                )
        nc.sync.drain()
        nc.all_engine_barrier()
        assert self.sems is not None
        popped = nc._tile_sem_poison_stack.pop()
        assert popped is self._sem_poison
        nc.clear_and_free_semaphores(list(self.sems.allocated().values()))
        nc.all_engine_barrier()


def _split_waits(nc, maxw=1):
    """The pinned walrus rejects instructions carrying more than one
    embedded sync wait ("Too many sync wait commands").  Hoist excess waits
    onto freshly inserted same-engine nops placed directly before the
    instruction — the engine sequencer executes them in order, so the
    semantics are identical."""
    for fn in nc.m.functions:
        for bb in fn.blocks:
            new = []
            changed = False
            for inst in bb.instructions:
                si = inst.sync_info
                waits = list(si.on_wait) if si is not None else []
                if len(waits) > maxw:
                    changed = True
                    n_extra = len(waits) - maxw
                    for i in range(0, n_extra, maxw):
                        nop = mybir.InstNoOp(
                            name=nc.get_next_instruction_name(),
                            engine=inst.engine,
                            sync_info=mybir.SyncInfo(
                                on_wait=waits[i:i + maxw], on_update=[]
                            ),
                            bass_nofuse=True,
                        )
                        nc.register_instruction(nop, overwrite=True)
                        new.append(nop)
                    si.on_wait = waits[n_extra:]
                new.append(inst)
            if changed:
                bb.instructions = new


def _token_tiles(C):
    # Remainder tile last: the first (full) tile's FFN1 masks the W2 load.
    tiles = [512] * (C // 512)
    if C % 512:
        tiles.append(C % 512)
    return tiles


def _build(C):
    """Dense per-expert FFN over C tokens; one SPMD program for all cores."""
    KH = H // 128   # 8  k-tiles over hidden
    KI = I // 128   # 32 k-tiles over inter
    nc = bass.Bass()
    xt = nc.declare_dram_parameter("xt", [H, C], BF16, isOutput=False)
    w1t = nc.declare_dram_parameter("w1t", [H, I], BF16, isOutput=False)
    w2t = nc.declare_dram_parameter("w2t", [I, H], BF16, isOutput=False)
    b1 = nc.declare_dram_parameter("b1", [128, KI], F32, isOutput=False)
    b2 = nc.declare_dram_parameter("b2", [128, KH], F32, isOutput=False)
    yt = nc.declare_dram_parameter("yt", [H, C], F32, isOutput=True)

    tiles = _token_tiles(C)
    offs = [sum(tiles[:i]) for i in range(len(tiles))]

    with _TC(nc) as tc:
        with (
            tc.tile_pool(name="weights", bufs=1) as wpool,
            tc.tile_pool(name="bias", bufs=1) as bpool,
            tc.tile_pool(name="x", bufs=3) as xpool,
            tc.tile_pool(name="h", bufs=1) as hpool,
            tc.tile_pool(name="o", bufs=4) as opool,
            tc.tile_pool(name="ps1", bufs=4, space="PSUM") as ps1pool,
            tc.tile_pool(name="ps2", bufs=4, space="PSUM") as ps2pool,
        ):
            # Every dma_start costs its issuing ENGINE ~600ns of dispatch
            # time, so transfers are batched into single wide-AP DMAs and
            # almost all dispatch lands on the otherwise-idle SP queue.
            # The Act queue carries only the latency-critical startup set
            # (biases + first x tile) and the per-tile x prefetch - its
            # engine time belongs to the gelu stream.
            b1s = bpool.tile([128, KI], F32, tag="b1")
            nc.scalar.dma_start(b1s[:], b1[:])
            b2s = bpool.tile([128, KH], F32, tag="b2")
            nc.scalar.dma_start(b2s[:], b2[:])

            xss = {}

            def emit_x(ti, eng):
                xs = xpool.tile([128, KH, tiles[ti]], BF16, tag="xt",
                                name=f"xs_{ti}")
                tw, off = tiles[ti], offs[ti]
                eng.dma_start(
                    xs[:, :, :],
                    xt[:, off:off + tw].rearrange("(k p) c -> p k c", p=128),
                )
                xss[ti] = xs

            emit_x(0, nc.scalar)

            # W1 in column phases on SP: a small first phase unblocks the
            # first psum groups, then three wide phases stay ahead of the
            # 256 KB / 1.7us consumption rate.  One dma_start per phase.
            w1s = wpool.tile([128, KH, I], BF16, tag="w1")
            bounds = [0, 256, 1536, 2816, I]
            for lo, hi in zip(bounds[:-1], bounds[1:]):
                nc.sync.dma_start(
                    w1s[:, :, lo:hi],
                    w1t[:, lo:hi].rearrange("(k p) c -> p k c", p=128),
                )
            # W2 afterwards on SP, in FFN2 consumption order (k ascending).
            w2s = wpool.tile([128, KI, H], BF16, tag="w2")
            for k0 in range(0, KI, 8):
                nc.sync.dma_start(
                    w2s[:, k0:k0 + 8, :],
                    w2t[k0 * 128:(k0 + 8) * 128, :].rearrange(
                        "(k p) c -> p k c", p=128),
                )

            for ti, tw in enumerate(tiles):
                off = offs[ti]
                xs = xss[ti]
                ht = hpool.tile([128, KI * tw], BF16, tag="h")
                for m in range(KI):
                    # Prefetch the next tile's x mid-FFN1: the Act engine
                    # reaches these dma_starts ~27us into this tile, far
                    # ahead of the next tile's first psum group.
                    if m == KI // 2 and ti + 1 < len(tiles):
                        emit_x(ti + 1, nc.scalar)
                    ps = ps1pool.tile([128, tw], F32, tag="ps1")
                    for k in range(KH):
                        nc.tensor.matmul(
                            ps[:],
                            w1s[:, k, m * 128:(m + 1) * 128],
                            xs[:, k, :],
                            start=(k == 0),
                            stop=(k == KH - 1),
                        )
                    nc.scalar.activation(
                        ht[:, m * tw:(m + 1) * tw],
                        ps[:],
                        mybir.ActivationFunctionType.Gelu,
                        bias=b1s[:, m:m + 1],
                    )
                for m in range(KH):
                    ps = ps2pool.tile([128, tw], F32, tag="ps2")
                    for k in range(KI):
                        nc.tensor.matmul(
                            ps[:],
                            w2s[:, k, m * 128:(m + 1) * 128],
                            ht[:, k * tw:(k + 1) * tw],
                            start=(k == 0),
                            stop=(k == KI - 1),
                        )
                    ot = opool.tile([128, tw], F32, tag="o")
                    nc.vector.tensor_scalar_add(ot[:], ps[:], b2s[:, m:m + 1])
                    nc.sync.dma_start(
                        yt[m * 128:(m + 1) * 128, off:off + tw], ot[:]
                    )
    _split_waits(nc)
    return nc


def _route(x, gate_w):
    """Host gate: top-2 of 8 logits + softmax over the selected pair."""
    logits = x @ gate_w.T                         # [T, E] f32
    T = logits.shape[0]
    rows = np.arange(T)
    i1 = np.argmax(logits, axis=1)
    v1 = logits[rows, i1]
    masked = logits.copy()
    masked[rows, i1] = -np.inf
    i2 = np.argmax(masked, axis=1)
    v2 = masked[rows, i2]
    # softmax over (v1, v2) with v1 >= v2
    e2 = np.exp(v2 - v1)
    w1 = 1.0 / (1.0 + e2)
    w2 = 1.0 - w1
    return i1, i2, w1.astype(np.float32), w2.astype(np.float32)


def _run(inputs, trace=False):
    hidden_states = np.asarray(inputs["hidden_states"], dtype=np.float32)
    gate_w = np.asarray(inputs["gate_w"], dtype=np.float32)
    W1 = np.asarray(inputs["W1"], dtype=np.float32)
    b1 = np.asarray(inputs["b1"], dtype=np.float32)
    W2 = np.asarray(inputs["W2"], dtype=np.float32)
    b2 = np.asarray(inputs["b2"], dtype=np.float32)

    B, S, _ = hidden_states.shape
    T = B * S
    x = np.ascontiguousarray(hidden_states.reshape(T, H))

    i1, i2, w1, w2 = _route(x, gate_w)
    toks = [np.flatnonzero((i1 == e) | (i2 == e)) for e in range(E)]
    cnts = [len(t) for t in toks]
    C = max(128, -(-max(cnts) // 128) * 128)

    nc = _build(C)

    in_maps = []
    for e in range(E):
        xe = np.zeros((C, H), dtype=ml_dtypes.bfloat16)
        xe[: cnts[e]] = x[toks[e]].astype(ml_dtypes.bfloat16)
        in_maps.append(
            {
                "xt": np.ascontiguousarray(xe.T),
                "w1t": np.ascontiguousarray(W1[e].astype(ml_dtypes.bfloat16).T),
                "w2t": np.ascontiguousarray(W2[e].astype(ml_dtypes.bfloat16).T),
                "b1": np.ascontiguousarray(b1[e].reshape(I // 128, 128).T),
                "b2": np.ascontiguousarray(b2[e].reshape(H // 128, 128).T),
            }
        )

    res = run_bass_kernel_spmd(
        nc, in_maps, core_ids=list(range(NCORES)), trace=trace
    )

    out = np.zeros((T, H), dtype=np.float32)
    for e in range(E):
        te = toks[e]
        ye = res.results[e]["yt"][:, : cnts[e]].T          # [cnt, H]
        we = np.where(i1[te] == e, w1[te], w2[te])
        out[te] += we[:, None] * ye
    return out.reshape(B, S, H), res


def kernel(**inputs):
    out, _ = _run(inputs, trace=False)
    return out



# revision 10
# speedup vs baseline: 1.5123x; 1.0060x over previous
"""MoE FFN (8 experts, top-2) on 8 Trainium2 NeuronCores.

Strategy: expert parallelism with host-side token routing.
  - Host computes the (tiny) gate: logits = x @ gate_w.T, top-2, softmax.
  - Tokens are gathered per expert and padded to a common capacity C.
  - Core e runs a dense FFN (gelu(x@W1[e].T+b1[e])@W2[e].T+b2[e]) over the
    C tokens routed to expert e, all in one SPMD Bass program.
  - Host scatters y back with the combine weights and sums the two
    expert contributions per token.

Device kernel layout (per core):
  FFN1: psum[inter128, tok] += W1T[k*128:, m*128:].T @ xT[k*128:, tok]
        h = gelu(psum + b1)           (ACT, writes bf16)
  FFN2: psum[hid128, tok]  += W2T[k*128:, m*128:].T @ h[k*128:, tok]
        y = psum + b2                 (DVE, writes f32)
Weights held resident in SBUF as bf16; tokens stream in tiles of <=512.
"""

import sys
import types

import numpy as np
import ml_dtypes

import concourse.bass as bass
import concourse.tile as tile
from concourse import mybir
from concourse.bass_utils import run_bass_kernel_spmd
from bass_rust import ScopedClock, VectorClock


def _ensure_axon_hooks():
    """run_bass_kernel_spmd(trace=True) under axon imports antenv.axon_hooks,
    which this image's antenv lacks.  Register an equivalent module backed by
    trn_agent_boot's ctypes NTFF hook so tracing works (and trace=False paths
    are unaffected)."""
    try:
        import antenv.axon_hooks  # noqa: F401
        return
    except ImportError:
        pass
    hook = None
    try:
        from trn_agent_boot.trn_boot import _ntff_profile_via_ctypes
        hook = _ntff_profile_via_ctypes("/opt/axon/libaxon_pjrt.so")
    except Exception:
        hook = None
    mod = types.ModuleType("antenv.axon_hooks")
    _state = {"hook": hook}
    mod.get_axon_ntff_profile_hook = lambda: _state["hook"]
    mod.set_axon_ntff_profile_hook = lambda h: _state.__setitem__("hook", h)
    sys.modules["antenv.axon_hooks"] = mod
    try:
        import antenv
        antenv.axon_hooks = mod
    except ImportError:
        pass


_ensure_axon_hooks()

H = 1024          # hidden
I = 4096          # intermediate
E = 8             # experts
NCORES = 8
BF16 = mybir.dt.bfloat16
F32 = mybir.dt.float32


class _TC(tile.TileContext):
    """TileContext whose tail drain splits its sem waits across SP nops.

    The walrus pinned in this container rejects a Drain instruction carrying
    more than a couple of sync waits ("Too many sync wait commands",
    CoreV3GenImpl.cpp:104).  Emit one wait-carrier nop per logical processor
    instead, then a waitless drain.
    """

    def _drain_and_barrier(self, tick_clock, wait_clock):
        nc = self.nc
        gc = tick_clock.global_clock
        ticks = eval(repr(gc).replace("VectorClock(", "").rstrip(")"))
        for i, t in enumerate(ticks):
            if t > 0:
                partial = [0] * len(ticks)
                partial[i] = t
                carrier = nc.sync.nop(nofuse=True, hint=f"drain_wait_{i}")
                wait_clock.add_sem_waits(
                    carrier.ins, ScopedClock({None: VectorClock(partial)})
                )
        nc.sync.drain()
        nc.all_engine_barrier()
        assert self.sems is not None
        popped = nc._tile_sem_poison_stack.pop()
        assert popped is self._sem_poison
        nc.clear_and_free_semaphores(list(self.sems.allocated().values()))
        nc.all_engine_barrier()


def _split_waits(nc, maxw=1):
    """The pinned walrus rejects instructions carrying more than one
    embedded sync wait ("Too many sync wait commands").  Hoist excess waits
    onto freshly inserted same-engine nops placed directly before the
    instruction — the engine sequencer executes them in order, so the
    semantics are identical."""
    for fn in nc.m.functions:
        for bb in fn.blocks:
            new = []
            changed = False
            for inst in bb.instructions:
                si = inst.sync_info
                waits = list(si.on_wait) if si is not None else []
                if len(waits) > maxw:
                    changed = True
                    n_extra = len(waits) - maxw
                    for i in range(0, n_extra, maxw):
                        nop = mybir.InstNoOp(
                            name=nc.get_next_instruction_name(),
                            engine=inst.engine,
                            sync_info=mybir.SyncInfo(
                                on_wait=waits[i:i + maxw], on_update=[]
                            ),
                            bass_nofuse=True,
                        )
                        nc.register_instruction(nop, overwrite=True)
                        new.append(nop)
                    si.on_wait = waits[n_extra:]
                new.append(inst)
            if changed:
                bb.instructions = new


def _token_tiles(C):
    # Remainder tile last: the first (full) tile's FFN1 masks the W2 load.
    tiles = [512] * (C // 512)
    if C % 512:
        tiles.append(C % 512)
    return tiles


def _build(C):
    """Dense per-expert FFN over C tokens; one SPMD program for all cores."""
    KH = H // 128   # 8  k-tiles over hidden
    KI = I // 128   # 32 k-tiles over inter
    nc = bass.Bass()
    xt = nc.declare_dram_parameter("xt", [H, C], BF16, isOutput=False)
    w1t = nc.declare_dram_parameter("w1t", [H, I], BF16, isOutput=False)
    w2t = nc.declare_dram_parameter("w2t", [I, H], BF16, isOutput=False)
    b1 = nc.declare_dram_parameter("b1", [128, KI], F32, isOutput=False)
    b2 = nc.declare_dram_parameter("b2", [128, KH], F32, isOutput=False)
    yt = nc.declare_dram_parameter("yt", [H, C], F32, isOutput=True)

    tiles = _token_tiles(C)
    offs = [sum(tiles[:i]) for i in range(len(tiles))]

    with _TC(nc) as tc:
        with (
            tc.tile_pool(name="weights", bufs=1) as wpool,
            tc.tile_pool(name="bias", bufs=1) as bpool,
            tc.tile_pool(name="x", bufs=3) as xpool,
            tc.tile_pool(name="h", bufs=1) as hpool,
            tc.tile_pool(name="o", bufs=4) as opool,
            tc.tile_pool(name="ps1", bufs=4, space="PSUM") as ps1pool,
            tc.tile_pool(name="ps2", bufs=4, space="PSUM") as ps2pool,
        ):
            # Every dma_start costs its issuing ENGINE ~600ns of dispatch
            # time, so transfers are batched into single wide-AP DMAs and
            # almost all dispatch lands on the otherwise-idle SP queue.
            # The Act queue carries only the latency-critical startup set
            # (biases + first x tile) and the per-tile x prefetch - its
            # engine time belongs to the gelu stream.
            b1s = bpool.tile([128, KI], F32, tag="b1")
            nc.scalar.dma_start(b1s[:], b1[:])
            b2s = bpool.tile([128, KH], F32, tag="b2")
            nc.scalar.dma_start(b2s[:], b2[:])

            xss = {}

            def emit_x(ti, eng, nchunk=1):
                xs = xpool.tile([128, KH, tiles[ti]], BF16, tag="xt",
                                name=f"xs_{ti}")
                tw, off = tiles[ti], offs[ti]
                kstep = KH // nchunk
                for k0 in range(0, KH, kstep):
                    eng.dma_start(
                        xs[:, k0:k0 + kstep, :],
                        xt[k0 * 128:(k0 + kstep) * 128, off:off + tw]
                        .rearrange("(k p) c -> p k c", p=128),
                    )
                xss[ti] = xs

            # Startup-critical transfers in several chunks so the HWDGE
            # spreads them across channels (a single big strided DMA runs
            # ~100 GB/s on one channel).
            emit_x(0, nc.scalar, nchunk=4)

            # W1 in column phases on SP: a small first phase unblocks the
            # first psum groups, then three wide phases stay ahead of the
            # 256 KB / 1.7us consumption rate.
            w1s = wpool.tile([128, KH, I], BF16, tag="w1")

            def w1_phase(lo, hi, ksplit=1):
                kstep = KH // ksplit
                for k0 in range(0, KH, kstep):
                    nc.sync.dma_start(
                        w1s[:, k0:k0 + kstep, lo:hi],
                        w1t[k0 * 128:(k0 + kstep) * 128, lo:hi]
                        .rearrange("(k p) c -> p k c", p=128),
                    )

            w1_phase(0, 256, ksplit=4)
            for lo, hi in [(256, 1536), (1536, 2816), (2816, I)]:
                w1_phase(lo, hi)
            # W2 afterwards on SP, in FFN2 consumption order (k ascending).
            w2s = wpool.tile([128, KI, H], BF16, tag="w2")
            for k0 in range(0, KI, 8):
                nc.sync.dma_start(
                    w2s[:, k0:k0 + 8, :],
                    w2t[k0 * 128:(k0 + 8) * 128, :].rearrange(
                        "(k p) c -> p k c", p=128),
                )

            for ti, tw in enumerate(tiles):
                off = offs[ti]
                xs = xss[ti]
                ht = hpool.tile([128, KI * tw], BF16, tag="h")
                for m in range(KI):
                    # Prefetch the next tile's x mid-FFN1: the Act engine
                    # reaches these dma_starts ~27us into this tile, far
                    # ahead of the next tile's first psum group.
                    if m == KI // 2 and ti + 1 < len(tiles):
                        emit_x(ti + 1, nc.scalar)
                    ps = ps1pool.tile([128, tw], F32, tag="ps1")
                    for k in range(KH):
                        nc.tensor.matmul(
                            ps[:],
                            w1s[:, k, m * 128:(m + 1) * 128],
                            xs[:, k, :],
                            start=(k == 0),
                            stop=(k == KH - 1),
                        )
                    nc.scalar.activation(
                        ht[:, m * tw:(m + 1) * tw],
                        ps[:],
                        mybir.ActivationFunctionType.Gelu,
                        bias=b1s[:, m:m + 1],
                    )
                for m in range(KH):
                    ps = ps2pool.tile([128, tw], F32, tag="ps2")
                    for k in range(KI):
                        nc.tensor.matmul(
                            ps[:],
                            w2s[:, k, m * 128:(m + 1) * 128],
                            ht[:, k * tw:(k + 1) * tw],
                            start=(k == 0),
                            stop=(k == KI - 1),
                        )
                    ot = opool.tile([128, tw], F32, tag="o")
                    nc.vector.tensor_scalar_add(ot[:], ps[:], b2s[:, m:m + 1])
                    nc.sync.dma_start(
                        yt[m * 128:(m + 1) * 128, off:off + tw], ot[:]
                    )
    _split_waits(nc)
    return nc


def _route(x, gate_w):
    """Host gate: top-2 of 8 logits + softmax over the selected pair."""
    logits = x @ gate_w.T                         # [T, E] f32
    T = logits.shape[0]
    rows = np.arange(T)
    i1 = np.argmax(logits, axis=1)
    v1 = logits[rows, i1]
    masked = logits.copy()
    masked[rows, i1] = -np.inf
    i2 = np.argmax(masked, axis=1)
    v2 = masked[rows, i2]
    # softmax over (v1, v2) with v1 >= v2
    e2 = np.exp(v2 - v1)
    w1 = 1.0 / (1.0 + e2)
    w2 = 1.0 - w1
    return i1, i2, w1.astype(np.float32), w2.astype(np.float32)


def _run(inputs, trace=False):
    hidden_states = np.asarray(inputs["hidden_states"], dtype=np.float32)
    gate_w = np.asarray(inputs["gate_w"], dtype=np.float32)
    W1 = np.asarray(inputs["W1"], dtype=np.float32)
    b1 = np.asarray(inputs["b1"], dtype=np.float32)
    W2 = np.asarray(inputs["W2"], dtype=np.float32)
    b2 = np.asarray(inputs["b2"], dtype=np.float32)

    B, S, _ = hidden_states.shape
    T = B * S
    x = np.ascontiguousarray(hidden_states.reshape(T, H))

    i1, i2, w1, w2 = _route(x, gate_w)
    toks = [np.flatnonzero((i1 == e) | (i2 == e)) for e in range(E)]
    cnts = [len(t) for t in toks]
    C = max(128, -(-max(cnts) // 128) * 128)

    nc = _build(C)

    in_maps = []
    for e in range(E):
        xe = np.zeros((C, H), dtype=ml_dtypes.bfloat16)
        xe[: cnts[e]] = x[toks[e]].astype(ml_dtypes.bfloat16)
        in_maps.append(
            {
                "xt": np.ascontiguousarray(xe.T),
                "w1t": np.ascontiguousarray(W1[e].astype(ml_dtypes.bfloat16).T),
                "w2t": np.ascontiguousarray(W2[e].astype(ml_dtypes.bfloat16).T),
                "b1": np.ascontiguousarray(b1[e].reshape(I // 128, 128).T),
                "b2": np.ascontiguousarray(b2[e].reshape(H // 128, 128).T),
            }
        )

    res = run_bass_kernel_spmd(
        nc, in_maps, core_ids=list(range(NCORES)), trace=trace
    )

    out = np.zeros((T, H), dtype=np.float32)
    for e in range(E):
        te = toks[e]
        ye = res.results[e]["yt"][:, : cnts[e]].T          # [cnt, H]
        we = np.where(i1[te] == e, w1[te], w2[te])
        out[te] += we[:, None] * ye
    return out.reshape(B, S, H), res


def kernel(**inputs):
    out, _ = _run(inputs, trace=False)
    return out



# revision 13
# speedup vs baseline: 1.5345x; 1.0147x over previous
"""MoE FFN (8 experts, top-2) on 8 Trainium2 NeuronCores.

Strategy: expert parallelism with host-side token routing.
  - Host computes the (tiny) gate: logits = x @ gate_w.T, top-2, softmax.
  - Tokens are gathered per expert and padded to a common capacity C.
  - Core e runs a dense FFN (gelu(x@W1[e].T+b1[e])@W2[e].T+b2[e]) over the
    C tokens routed to expert e, all in one SPMD Bass program.
  - Host scatters y back with the combine weights and sums the two
    expert contributions per token.

Device kernel layout (per core):
  FFN1: psum[inter128, tok] += W1T[k*128:, m*128:].T @ xT[k*128:, tok]
        h = gelu(psum + b1)           (ACT, writes bf16)
  FFN2: psum[hid128, tok]  += W2T[k*128:, m*128:].T @ h[k*128:, tok]
        y = psum + b2                 (DVE, writes f32)
Weights held resident in SBUF as bf16; tokens stream in tiles of <=512.
"""

import sys
import types

import numpy as np
import ml_dtypes

import concourse.bass as bass
import concourse.tile as tile
from concourse import mybir
from concourse.bass_utils import run_bass_kernel_spmd
from bass_rust import ScopedClock, VectorClock


def _ensure_axon_hooks():
    """run_bass_kernel_spmd(trace=True) under axon imports antenv.axon_hooks,
    which this image's antenv lacks.  Register an equivalent module backed by
    trn_agent_boot's ctypes NTFF hook so tracing works (and trace=False paths
    are unaffected)."""
    try:
        import antenv.axon_hooks  # noqa: F401
        return
    except ImportError:
        pass
    hook = None
    try:
        from trn_agent_boot.trn_boot import _ntff_profile_via_ctypes
        hook = _ntff_profile_via_ctypes("/opt/axon/libaxon_pjrt.so")
    except Exception:
        hook = None
    mod = types.ModuleType("antenv.axon_hooks")
    _state = {"hook": hook}
    mod.get_axon_ntff_profile_hook = lambda: _state["hook"]
    mod.set_axon_ntff_profile_hook = lambda h: _state.__setitem__("hook", h)
    sys.modules["antenv.axon_hooks"] = mod
    try:
        import antenv
        antenv.axon_hooks = mod
    except ImportError:
        pass


_ensure_axon_hooks()

H = 1024          # hidden
I = 4096          # intermediate
E = 8             # experts
NCORES = 8
BF16 = mybir.dt.bfloat16
F32 = mybir.dt.float32


class _TC(tile.TileContext):
    """TileContext whose tail drain splits its sem waits across SP nops.

    The walrus pinned in this container rejects a Drain instruction carrying
    more than a couple of sync waits ("Too many sync wait commands",
    CoreV3GenImpl.cpp:104).  Emit one wait-carrier nop per logical processor
    instead, then a waitless drain.
    """

    def _drain_and_barrier(self, tick_clock, wait_clock):
        nc = self.nc
        gc = tick_clock.global_clock
        ticks = eval(repr(gc).replace("VectorClock(", "").rstrip(")"))
        for i, t in enumerate(ticks):
            if t > 0:
                partial = [0] * len(ticks)
                partial[i] = t
                carrier = nc.sync.nop(nofuse=True, hint=f"drain_wait_{i}")
                wait_clock.add_sem_waits(
                    carrier.ins, ScopedClock({None: VectorClock(partial)})
                )
        nc.sync.drain()
        nc.all_engine_barrier()
        assert self.sems is not None
        popped = nc._tile_sem_poison_stack.pop()
        assert popped is self._sem_poison
        # The ~60 serialized EVENT_SEMAPHORE clears cost ~8us of tail.
        # Each kernel() invocation compiles and executes a fresh NEFF
        # exactly once, so end-state semaphore values are never observed;
        # free the handles bass-side without emitting clear instructions.
        for s in self.sems.allocated().values():
            nc.release_semaphore(s)
        nc.all_engine_barrier()


def _split_waits(nc, maxw=1):
    """The pinned walrus rejects instructions carrying more than one
    embedded sync wait ("Too many sync wait commands").  Hoist excess waits
    onto freshly inserted same-engine nops placed directly before the
    instruction — the engine sequencer executes them in order, so the
    semantics are identical."""
    for fn in nc.m.functions:
        for bb in fn.blocks:
            new = []
            changed = False
            for inst in bb.instructions:
                si = inst.sync_info
                waits = list(si.on_wait) if si is not None else []
                if len(waits) > maxw:
                    changed = True
                    n_extra = len(waits) - maxw
                    for i in range(0, n_extra, maxw):
                        nop = mybir.InstNoOp(
                            name=nc.get_next_instruction_name(),
                            engine=inst.engine,
                            sync_info=mybir.SyncInfo(
                                on_wait=waits[i:i + maxw], on_update=[]
                            ),
                            bass_nofuse=True,
                        )
                        nc.register_instruction(nop, overwrite=True)
                        new.append(nop)
                    si.on_wait = waits[n_extra:]
                new.append(inst)
            if changed:
                bb.instructions = new


def _token_tiles(C):
    # Remainder tile last: the first (full) tile's FFN1 masks the W2 load.
    tiles = [512] * (C // 512)
    if C % 512:
        tiles.append(C % 512)
    return tiles


def _build(C):
    """Dense per-expert FFN over C tokens; one SPMD program for all cores."""
    KH = H // 128   # 8  k-tiles over hidden
    KI = I // 128   # 32 k-tiles over inter
    nc = bass.Bass()
    xt = nc.declare_dram_parameter("xt", [H, C], BF16, isOutput=False)
    w1t = nc.declare_dram_parameter("w1t", [H, I], BF16, isOutput=False)
    w2t = nc.declare_dram_parameter("w2t", [I, H], BF16, isOutput=False)
    b1 = nc.declare_dram_parameter("b1", [128, KI], F32, isOutput=False)
    b2 = nc.declare_dram_parameter("b2", [128, KH], F32, isOutput=False)
    yt = nc.declare_dram_parameter("yt", [H, C], F32, isOutput=True)

    tiles = _token_tiles(C)
    offs = [sum(tiles[:i]) for i in range(len(tiles))]

    with _TC(nc) as tc:
        with (
            tc.tile_pool(name="weights", bufs=1) as wpool,
            tc.tile_pool(name="bias", bufs=1) as bpool,
            tc.tile_pool(name="x", bufs=3) as xpool,
            tc.tile_pool(name="h", bufs=1) as hpool,
            tc.tile_pool(name="o", bufs=4) as opool,
            tc.tile_pool(name="ps1", bufs=4, space="PSUM") as ps1pool,
            tc.tile_pool(name="ps2", bufs=4, space="PSUM") as ps2pool,
        ):
            # Every dma_start costs its issuing ENGINE ~600ns of dispatch
            # time, so transfers are batched into single wide-AP DMAs and
            # almost all dispatch lands on the otherwise-idle SP queue.
            # The Act queue carries only the latency-critical startup set
            # (biases + first x tile) and the per-tile x prefetch - its
            # engine time belongs to the gelu stream.
            b1s = bpool.tile([128, KI], F32, tag="b1")
            nc.scalar.dma_start(b1s[:], b1[:])
            b2s = bpool.tile([128, KH], F32, tag="b2")
            nc.scalar.dma_start(b2s[:], b2[:])

            xss = {}

            def emit_x(ti, eng, nchunk=1):
                xs = xpool.tile([128, KH, tiles[ti]], BF16, tag="xt",
                                name=f"xs_{ti}")
                tw, off = tiles[ti], offs[ti]
                kstep = KH // nchunk
                for k0 in range(0, KH, kstep):
                    eng.dma_start(
                        xs[:, k0:k0 + kstep, :],
                        xt[k0 * 128:(k0 + kstep) * 128, off:off + tw]
                        .rearrange("(k p) c -> p k c", p=128),
                    )
                xss[ti] = xs

            # Startup-critical transfers in several chunks so the HWDGE
            # spreads them across channels (a single big strided DMA runs
            # ~100 GB/s on one channel).
            emit_x(0, nc.scalar, nchunk=4)

            # W1 in column phases on SP: a small first phase unblocks the
            # first psum groups, then three wide phases stay ahead of the
            # 256 KB / 1.7us consumption rate.
            w1s = wpool.tile([128, KH, I], BF16, tag="w1")

            def w1_phase(lo, hi, ksplit=1):
                kstep = KH // ksplit
                for k0 in range(0, KH, kstep):
                    nc.sync.dma_start(
                        w1s[:, k0:k0 + kstep, lo:hi],
                        w1t[k0 * 128:(k0 + kstep) * 128, lo:hi]
                        .rearrange("(k p) c -> p k c", p=128),
                    )

            w1_phase(0, 256, ksplit=4)
            for lo, hi in [(256, 512), (512, 1024), (1024, 1536),
                           (1536, 2304), (2304, 3072), (3072, I)]:
                w1_phase(lo, hi, ksplit=2)
            # W2 afterwards on SP, in FFN2 consumption order (k ascending).
            w2s = wpool.tile([128, KI, H], BF16, tag="w2")
            for k0 in range(0, KI, 8):
                nc.sync.dma_start(
                    w2s[:, k0:k0 + 8, :],
                    w2t[k0 * 128:(k0 + 8) * 128, :].rearrange(
                        "(k p) c -> p k c", p=128),
                )

            for ti, tw in enumerate(tiles):
                off = offs[ti]
                xs = xss[ti]
                ht = hpool.tile([128, KI * tw], BF16, tag="h")
                for m in range(KI):
                    # Prefetch the next tile's x mid-FFN1: the Act engine
                    # reaches these dma_starts ~27us into this tile, far
                    # ahead of the next tile's first psum group.
                    if m == KI // 2 and ti + 1 < len(tiles):
                        emit_x(ti + 1, nc.scalar)
                    ps = ps1pool.tile([128, tw], F32, tag="ps1")
                    for k in range(KH):
                        nc.tensor.matmul(
                            ps[:],
                            w1s[:, k, m * 128:(m + 1) * 128],
                            xs[:, k, :],
                            start=(k == 0),
                            stop=(k == KH - 1),
                        )
                    nc.scalar.activation(
                        ht[:, m * tw:(m + 1) * tw],
                        ps[:],
                        mybir.ActivationFunctionType.Gelu,
                        bias=b1s[:, m:m + 1],
                    )
                for m in range(KH):
                    ps = ps2pool.tile([128, tw], F32, tag="ps2")
                    for k in range(KI):
                        nc.tensor.matmul(
                            ps[:],
                            w2s[:, k, m * 128:(m + 1) * 128],
                            ht[:, k * tw:(k + 1) * tw],
                            start=(k == 0),
                            stop=(k == KI - 1),
                        )
                    ot = opool.tile([128, tw], F32, tag="o")
                    nc.vector.tensor_scalar_add(ot[:], ps[:], b2s[:, m:m + 1])
                    nc.sync.dma_start(
                        yt[m * 128:(m + 1) * 128, off:off + tw], ot[:]
                    )
    _split_waits(nc)
    return nc


def _route(x, gate_w):
    """Host gate: top-2 of 8 logits + softmax over the selected pair."""
    logits = x @ gate_w.T                         # [T, E] f32
    T = logits.shape[0]
    rows = np.arange(T)
    i1 = np.argmax(logits, axis=1)
    v1 = logits[rows, i1]
    masked = logits.copy()
    masked[rows, i1] = -np.inf
    i2 = np.argmax(masked, axis=1)
    v2 = masked[rows, i2]
    # softmax over (v1, v2) with v1 >= v2
    e2 = np.exp(v2 - v1)
    w1 = 1.0 / (1.0 + e2)
    w2 = 1.0 - w1
    return i1, i2, w1.astype(np.float32), w2.astype(np.float32)


def _run(inputs, trace=False):
    hidden_states = np.asarray(inputs["hidden_states"], dtype=np.float32)
    gate_w = np.asarray(inputs["gate_w"], dtype=np.float32)
    W1 = np.asarray(inputs["W1"], dtype=np.float32)
    b1 = np.asarray(inputs["b1"], dtype=np.float32)
    W2 = np.asarray(inputs["W2"], dtype=np.float32)
    b2 = np.asarray(inputs["b2"], dtype=np.float32)

    B, S, _ = hidden_states.shape
    T = B * S
    x = np.ascontiguousarray(hidden_states.reshape(T, H))

    i1, i2, w1, w2 = _route(x, gate_w)
    toks = [np.flatnonzero((i1 == e) | (i2 == e)) for e in range(E)]
    cnts = [len(t) for t in toks]
    C = max(128, -(-max(cnts) // 128) * 128)

    nc = _build(C)

    in_maps = []
    for e in range(E):
        xe = np.zeros((C, H), dtype=ml_dtypes.bfloat16)
        xe[: cnts[e]] = x[toks[e]].astype(ml_dtypes.bfloat16)
        in_maps.append(
            {
                "xt": np.ascontiguousarray(xe.T),
                "w1t": np.ascontiguousarray(W1[e].astype(ml_dtypes.bfloat16).T),
                "w2t": np.ascontiguousarray(W2[e].astype(ml_dtypes.bfloat16).T),
                "b1": np.ascontiguousarray(b1[e].reshape(I // 128, 128).T),
                "b2": np.ascontiguousarray(b2[e].reshape(H // 128, 128).T),
            }
        )

    res = run_bass_kernel_spmd(
        nc, in_maps, core_ids=list(range(NCORES)), trace=trace
    )

    out = np.zeros((T, H), dtype=np.float32)
    for e in range(E):
        te = toks[e]
        ye = res.results[e]["yt"][:, : cnts[e]].T          # [cnt, H]
        we = np.where(i1[te] == e, w1[te], w2[te])
        out[te] += we[:, None] * ye
    return out.reshape(B, S, H), res


def kernel(**inputs):
    out, _ = _run(inputs, trace=False)
    return out



# revision 15
# speedup vs baseline: 1.5721x; 1.0245x over previous
"""MoE FFN (8 experts, top-2) on 8 Trainium2 NeuronCores.

Strategy: expert parallelism with host-side token routing.
  - Host computes the (tiny) gate: logits = x @ gate_w.T, top-2, softmax.
  - Tokens are gathered per expert and padded to a common capacity C.
  - Core e runs a dense FFN (gelu(x@W1[e].T+b1[e])@W2[e].T+b2[e]) over the
    C tokens routed to expert e, all in one SPMD Bass program.
  - Host scatters y back with the combine weights and sums the two
    expert contributions per token.

Device kernel layout (per core):
  FFN1: psum[inter128, tok] += W1T[k*128:, m*128:].T @ xT[k*128:, tok]
        h = gelu(psum + b1)           (ACT, writes bf16)
  FFN2: psum[hid128, tok]  += W2T[k*128:, m*128:].T @ h[k*128:, tok]
        y = psum + b2                 (DVE, writes f32)
Weights held resident in SBUF as bf16; tokens stream in tiles of <=512.
"""

import sys
import types

import numpy as np
import ml_dtypes

import concourse.bass as bass
import concourse.tile as tile
from concourse import mybir
from concourse.bass_utils import run_bass_kernel_spmd
from bass_rust import ScopedClock, VectorClock


def _ensure_axon_hooks():
    """run_bass_kernel_spmd(trace=True) under axon imports antenv.axon_hooks,
    which this image's antenv lacks.  Register an equivalent module backed by
    trn_agent_boot's ctypes NTFF hook so tracing works (and trace=False paths
    are unaffected)."""
    try:
        import antenv.axon_hooks  # noqa: F401
        return
    except ImportError:
        pass
    hook = None
    try:
        from trn_agent_boot.trn_boot import _ntff_profile_via_ctypes
        hook = _ntff_profile_via_ctypes("/opt/axon/libaxon_pjrt.so")
    except Exception:
        hook = None
    mod = types.ModuleType("antenv.axon_hooks")
    _state = {"hook": hook}
    mod.get_axon_ntff_profile_hook = lambda: _state["hook"]
    mod.set_axon_ntff_profile_hook = lambda h: _state.__setitem__("hook", h)
    sys.modules["antenv.axon_hooks"] = mod
    try:
        import antenv
        antenv.axon_hooks = mod
    except ImportError:
        pass


_ensure_axon_hooks()

H = 1024          # hidden
I = 4096          # intermediate
E = 8             # experts
NCORES = 8
BF16 = mybir.dt.bfloat16
F32 = mybir.dt.float32


class _TC(tile.TileContext):
    """TileContext whose tail drain splits its sem waits across SP nops.

    The walrus pinned in this container rejects a Drain instruction carrying
    more than a couple of sync waits ("Too many sync wait commands",
    CoreV3GenImpl.cpp:104).  Emit one wait-carrier nop per logical processor
    instead, then a waitless drain.
    """

    def _drain_and_barrier(self, tick_clock, wait_clock):
        nc = self.nc
        gc = tick_clock.global_clock
        ticks = eval(repr(gc).replace("VectorClock(", "").rstrip(")"))
        for i, t in enumerate(ticks):
            if t > 0:
                partial = [0] * len(ticks)
                partial[i] = t
                carrier = nc.sync.nop(nofuse=True, hint=f"drain_wait_{i}")
                wait_clock.add_sem_waits(
                    carrier.ins, ScopedClock({None: VectorClock(partial)})
                )
        nc.sync.drain()
        nc.all_engine_barrier()
        assert self.sems is not None
        popped = nc._tile_sem_poison_stack.pop()
        assert popped is self._sem_poison
        # The ~60 serialized EVENT_SEMAPHORE clears cost ~8us of tail.
        # Each kernel() invocation compiles and executes a fresh NEFF
        # exactly once, so end-state semaphore values are never observed;
        # free the handles bass-side without emitting clear instructions.
        for s in self.sems.allocated().values():
            nc.release_semaphore(s)
        nc.all_engine_barrier()


def _split_waits(nc, maxw=1):
    """The pinned walrus rejects instructions carrying more than one
    embedded sync wait ("Too many sync wait commands").  Hoist excess waits
    onto freshly inserted same-engine nops placed directly before the
    instruction — the engine sequencer executes them in order, so the
    semantics are identical."""
    for fn in nc.m.functions:
        for bb in fn.blocks:
            new = []
            changed = False
            for inst in bb.instructions:
                si = inst.sync_info
                waits = list(si.on_wait) if si is not None else []
                if len(waits) > maxw:
                    changed = True
                    n_extra = len(waits) - maxw
                    for i in range(0, n_extra, maxw):
                        nop = mybir.InstNoOp(
                            name=nc.get_next_instruction_name(),
                            engine=inst.engine,
                            sync_info=mybir.SyncInfo(
                                on_wait=waits[i:i + maxw], on_update=[]
                            ),
                            bass_nofuse=True,
                        )
                        nc.register_instruction(nop, overwrite=True)
                        new.append(nop)
                    si.on_wait = waits[n_extra:]
                new.append(inst)
            if changed:
                bb.instructions = new


def _token_tiles(C):
    # Remainder tile last: the first (full) tile's FFN1 masks the W2 load.
    tiles = [512] * (C // 512)
    if C % 512:
        tiles.append(C % 512)
    return tiles


I2 = I // 2       # intermediate half per unit
KH = H // 128     # 8  k-tiles over hidden
KI2 = I2 // 128   # 16 k/m-tiles over the intermediate half


def _build(Cs):
    """Two half-expert FFN units per core (load rebalance).

    Each unit u computes, over C_u tokens of one expert, the FFN restricted
    to one half of the intermediate dim:  partial_y = gelu(x W1h.T + b1h)
    @ W2h.T  (b2 is added on the host when the two halves are combined).
    Splitting along I keeps per-unit weights at 8 MB, so a core holds two
    units (16 MB) and the 8 largest token loads pair with the 8 smallest:
    per-core work drops from max_e C_e to (C_A + C_B) / 2.
    """
    nc = bass.Bass()
    xts, w1ts, w2ts, b1ts, yts = [], [], [], [], []
    for u, C in enumerate(Cs):
        s = "ab"[u]
        xts.append(nc.declare_dram_parameter(f"x{s}", [H, C], BF16,
                                             isOutput=False))
        w1ts.append(nc.declare_dram_parameter(f"w1{s}", [H, I2], BF16,
                                              isOutput=False))
        w2ts.append(nc.declare_dram_parameter(f"w2{s}", [I2, H], BF16,
                                              isOutput=False))
        b1ts.append(nc.declare_dram_parameter(f"b1{s}", [128, KI2], F32,
                                              isOutput=False))
        yts.append(nc.declare_dram_parameter(f"y{s}", [H, C], F32,
                                             isOutput=True))

    # Flat tile schedule across both units: (unit, tw, off)
    sched = []
    for u, C in enumerate(Cs):
        off = 0
        for tw in _token_tiles(C):
            sched.append((u, tw, off))
            off += tw

    with _TC(nc) as tc:
        with (
            tc.tile_pool(name="weights", bufs=1) as wpool,
            tc.tile_pool(name="bias", bufs=1) as bpool,
            tc.tile_pool(name="x", bufs=3) as xpool,
            tc.tile_pool(name="h", bufs=1) as hpool,
            tc.tile_pool(name="o", bufs=4) as opool,
            tc.tile_pool(name="ps1", bufs=4, space="PSUM") as ps1pool,
            tc.tile_pool(name="ps2", bufs=4, space="PSUM") as ps2pool,
        ):
            # Every dma_start costs its issuing ENGINE ~600ns of dispatch
            # time, so transfers are batched into wide-AP DMAs and almost
            # all dispatch lands on the otherwise-idle SP queue.  The Act
            # queue carries only the latency-critical startup set (biases +
            # first x tile) and the per-tile x prefetch.
            b1ss = []
            for u in range(2):
                b1s = bpool.tile([128, KI2], F32, tag=f"b1{u}",
                                 name=f"b1s{u}")
                nc.scalar.dma_start(b1s[:], b1ts[u][:])
                b1ss.append(b1s)

            xss = {}

            def emit_x(si, eng, nchunk=1):
                u, tw, off = sched[si]
                xs = xpool.tile([128, KH, tw], BF16, tag="xt",
                                name=f"xs_{si}")
                kstep = KH // nchunk
                for k0 in range(0, KH, kstep):
                    eng.dma_start(
                        xs[:, k0:k0 + kstep, :],
                        xts[u][k0 * 128:(k0 + kstep) * 128, off:off + tw]
                        .rearrange("(k p) c -> p k c", p=128),
                    )
                xss[si] = xs

            # Startup-critical transfers in several chunks so the HWDGE
            # spreads them across channels (one big strided DMA runs
            # ~100 GB/s on a single channel).
            emit_x(0, nc.scalar, nchunk=4)

            # Unit-a W1 in column phases on SP (small first phase unblocks
            # the first psum groups); then W2a, then unit-b weights coarse
            # (consumed only ~230us in).
            w1ss, w2ss = [], []
            for u in range(2):
                w1ss.append(wpool.tile([128, KH, I2], BF16, tag=f"w1{u}",
                                       name=f"w1s{u}"))
                w2ss.append(wpool.tile([128, KI2, H], BF16, tag=f"w2{u}",
                                       name=f"w2s{u}"))

            def w1_phase(u, lo, hi, ksplit=1):
                kstep = KH // ksplit
                for k0 in range(0, KH, kstep):
                    nc.sync.dma_start(
                        w1ss[u][:, k0:k0 + kstep, lo:hi],
                        w1ts[u][k0 * 128:(k0 + kstep) * 128, lo:hi]
                        .rearrange("(k p) c -> p k c", p=128),
                    )

            def w2_load(u, ksplit=2):
                kstep = KI2 // ksplit
                for k0 in range(0, KI2, kstep):
                    nc.sync.dma_start(
                        w2ss[u][:, k0:k0 + kstep, :],
                        w2ts[u][k0 * 128:(k0 + kstep) * 128, :]
                        .rearrange("(k p) c -> p k c", p=128),
                    )

            w1_phase(0, 0, 256, ksplit=4)
            for lo, hi in [(256, 512), (512, 1024), (1024, 1536),
                           (1536, I2)]:
                w1_phase(0, lo, hi, ksplit=2)
            w2_load(0)
            w1_phase(1, 0, I2, ksplit=2)
            w2_load(1)

            for si, (u, tw, off) in enumerate(sched):
                xs = xss[si]
                w1s, w2s, b1s, yt = w1ss[u], w2ss[u], b1ss[u], yts[u]
                ht = hpool.tile([128, KI2, tw], BF16, tag="h")
                for m in range(KI2):
                    # Prefetch upcoming x tiles mid-FFN1, far ahead of
                    # their first psum group.  When the next tile is the
                    # short unit tail, also prefetch the one after it.
                    if m == 8 and si + 1 < len(sched):
                        emit_x(si + 1, nc.scalar)
                    if (m == 12 and si + 2 < len(sched)
                            and sched[si + 1][1] <= 128):
                        emit_x(si + 2, nc.scalar)
                    ps = ps1pool.tile([128, tw], F32, tag="ps1")
                    for k in range(KH):
                        nc.tensor.matmul(
                            ps[:],
                            w1s[:, k, m * 128:(m + 1) * 128],
                            xs[:, k, :],
                            start=(k == 0),
                            stop=(k == KH - 1),
                        )
                    nc.scalar.activation(
                        ht[:, m, :],
                        ps[:],
                        mybir.ActivationFunctionType.Gelu,
                        bias=b1s[:, m:m + 1],
                    )
                for m in range(KH):
                    ps = ps2pool.tile([128, tw], F32, tag="ps2")
                    for k in range(KI2):
                        nc.tensor.matmul(
                            ps[:],
                            w2s[:, k, m * 128:(m + 1) * 128],
                            ht[:, k, :],
                            start=(k == 0),
                            stop=(k == KI2 - 1),
                        )
                    ot = opool.tile([128, tw], F32, tag="o")
                    nc.vector.tensor_copy(ot[:], ps[:])
                    nc.sync.dma_start(
                        yt[m * 128:(m + 1) * 128, off:off + tw], ot[:]
                    )
    _split_waits(nc)
    return nc


def _route(x, gate_w):
    """Host gate: top-2 of 8 logits + softmax over the selected pair."""
    logits = x @ gate_w.T                         # [T, E] f32
    T = logits.shape[0]
    rows = np.arange(T)
    i1 = np.argmax(logits, axis=1)
    v1 = logits[rows, i1]
    masked = logits.copy()
    masked[rows, i1] = -np.inf
    i2 = np.argmax(masked, axis=1)
    v2 = masked[rows, i2]
    # softmax over (v1, v2) with v1 >= v2
    e2 = np.exp(v2 - v1)
    w1 = 1.0 / (1.0 + e2)
    w2 = 1.0 - w1
    return i1, i2, w1.astype(np.float32), w2.astype(np.float32)


def _run(inputs, trace=False):
    hidden_states = np.asarray(inputs["hidden_states"], dtype=np.float32)
    gate_w = np.asarray(inputs["gate_w"], dtype=np.float32)
    W1 = np.asarray(inputs["W1"], dtype=np.float32)
    b1 = np.asarray(inputs["b1"], dtype=np.float32)
    W2 = np.asarray(inputs["W2"], dtype=np.float32)
    b2 = np.asarray(inputs["b2"], dtype=np.float32)

    B, S, _ = hidden_states.shape
    T = B * S
    x = np.ascontiguousarray(hidden_states.reshape(T, H))

    i1, i2, w1, w2 = _route(x, gate_w)
    toks = [np.flatnonzero((i1 == e) | (i2 == e)) for e in range(E)]
    cnts = [len(t) for t in toks]

    # 16 half-expert units (e, half) sorted by token count: A-slots get the
    # 8 heaviest, B-slots the 8 lightest; core i runs units[i] + units[8+i].
    units = sorted(
        [(e, hf) for e in range(E) for hf in range(2)],
        key=lambda u: -cnts[u[0]],
    )
    pad = lambda n: max(128, -(-n // 128) * 128)
    C_A = pad(max(cnts[e] for e, _ in units[:NCORES]))
    C_B = pad(max(cnts[e] for e, _ in units[NCORES:]))

    nc = _build([C_A, C_B])

    xes = {}
    for e in range(E):
        xe = np.zeros((max(C_A, C_B), H), dtype=ml_dtypes.bfloat16)
        xe[: cnts[e]] = x[toks[e]].astype(ml_dtypes.bfloat16)
        xes[e] = np.ascontiguousarray(xe.T)                # [H, Cmax]

    in_maps = []
    for core in range(NCORES):
        m = {}
        for u, C in ((0, C_A), (1, C_B)):
            e, hf = units[u * NCORES + core]
            s = "ab"[u]
            w1h = W1[e][hf * I2:(hf + 1) * I2, :]          # [I2, H]
            w2h = W2[e][:, hf * I2:(hf + 1) * I2]          # [H, I2]
            m[f"x{s}"] = np.ascontiguousarray(xes[e][:, :C])
            m[f"w1{s}"] = np.ascontiguousarray(
                w1h.astype(ml_dtypes.bfloat16).T)          # [H, I2]
            m[f"w2{s}"] = np.ascontiguousarray(
                w2h.astype(ml_dtypes.bfloat16).T)          # [I2, H]
            m[f"b1{s}"] = np.ascontiguousarray(
                b1[e][hf * I2:(hf + 1) * I2].reshape(KI2, 128).T)
        in_maps.append(m)

    res = run_bass_kernel_spmd(
        nc, in_maps, core_ids=list(range(NCORES)), trace=trace
    )

    # Sum each expert's two half-unit partials, add b2, scatter-combine.
    acc = {e: None for e in range(E)}
    for core in range(NCORES):
        for u in range(2):
            e, _ = units[u * NCORES + core]
            ye = res.results[core]["y" + "ab"[u]][:, : cnts[e]].T
            acc[e] = ye if acc[e] is None else acc[e] + ye
    out = np.zeros((T, H), dtype=np.float32)
    for e in range(E):
        te = toks[e]
        we = np.where(i1[te] == e, w1[te], w2[te])
        out[te] += we[:, None] * (acc[e] + b2[e])
    return out.reshape(B, S, H), res


def kernel(**inputs):
    out, _ = _run(inputs, trace=False)
    return out

